# revision 3
# baseline (speedup 1.0000x reference)
"""GraphSAGE-style GNN layer on 8 Trainium2 NeuronCores.

out = relu(W @ concat([features[nodes], mean(features[neigh_idx], 1)], 1).T)

Strategy: replicate the feature table + (host-transposed, mean-folded) weight
on all 8 cores; data-parallel over the 16384-node batch (2048 nodes/core).
Per 128-node tile: 11 indirect-DMA row gathers (self + 10 neighbors),
VectorE neighbor-sum, PE transposes to feature-major, fp32 matmul accumulated
over K=512 in PSUM, ScalarE ReLU, store [256, 2048] slice.
"""
import numpy as np

N_CORES = 8
NUM_NODES = 1_000_000
F = 256
E = 256
B = 16384
NSAMP = 10
SLOTS = 1 + NSAMP
P = 128
B_LOCAL = B // N_CORES          # 2048
TILES = B_LOCAL // P            # 16
GROUP = 4                       # b-tiles per matmul group (N=512)

_cache = {}


def _build():
    import concourse.bass as bass
    import concourse.bacc as bacc
    import concourse.mybir as mybir
    import concourse.tile as tile
    from concourse.masks import make_identity

    nc = bacc.Bacc("TRN2", target_bir_lowering=False, debug=False)
    feats = nc.dram_tensor("features", [NUM_NODES, F], mybir.dt.float32,
                           kind="ExternalInput")
    wt = nc.dram_tensor("wt", [2 * F, E], mybir.dt.float32, kind="ExternalInput")
    gidx = nc.dram_tensor("gidx", [B_LOCAL, SLOTS], mybir.dt.int32,
                          kind="ExternalInput")
    out = nc.dram_tensor("out", [E, B_LOCAL], mybir.dt.float32,
                         kind="ExternalOutput")

    with tile.TileContext(nc) as tc:
        with (
            tc.tile_pool(name="const", bufs=1) as constp,
            tc.tile_pool(name="gather", bufs=3) as gatherp,
            tc.tile_pool(name="acc", bufs=3) as accp,
            tc.tile_pool(name="combT", bufs=2) as combp,
            tc.tile_pool(name="outs", bufs=3) as outsp,
            tc.tile_pool(name="pst", bufs=4, space="PSUM") as pst,
            tc.tile_pool(name="psc", bufs=1, space="PSUM") as psc,
            tc.tile_pool(name="psm", bufs=2, space="PSUM") as psm,
        ):
            ident = constp.tile([P, P], mybir.dt.float32)
            make_identity(nc, ident[:])
            # absorb the identity-ready wait on PE (Matmult carries 1 HW wait)
            scratch = psc.tile([P, P], mybir.dt.float32, tag="scratch")
            nc.tensor.transpose(out=scratch[:], in_=ident[:], identity=ident[:])

            # weights: wtile[k, c*E+e] = wt[c*128+k, e]
            wtile = constp.tile([P, 4 * E], mybir.dt.float32)
            nc.sync.dma_start(
                out=wtile[:].rearrange("k (c e) -> k c e", c=4),
                in_=wt.ap().rearrange("(c k) e -> k c e", k=P),
            )
            # all indices: ixall[p, t*SLOTS+s] = gidx[t*128+p, s]
            ixall = constp.tile([P, TILES * SLOTS], mybir.dt.int32)
            nc.sync.dma_start(
                out=ixall[:].rearrange("p (t s) -> p t s", t=TILES),
                in_=gidx.ap().rearrange("(t p) s -> p t s", p=P),
            )

            combT = None
            for t in range(TILES):
                g = t // GROUP
                bt = t % GROUP
                if bt == 0:
                    combT = [combp.tile([P, GROUP * P], mybir.dt.float32,
                                        tag=f"combT{kc}", name=f"combT{kc}_{g}")
                             for kc in range(4)]
                # gather: slot tiles [128, 256], one row per partition
                slots = []
                for s in range(SLOTS):
                    gt = gatherp.tile([P, F], mybir.dt.float32, tag=f"slot{s}")
                    nc.gpsimd.indirect_dma_start(
                        out=gt[:], out_offset=None, in_=feats.ap()[:, :],
                        in_offset=bass.IndirectOffsetOnAxis(
                            ap=ixall[:, t * SLOTS + s: t * SLOTS + s + 1], axis=0),
                    )
                    slots.append(gt)
                # neighbor sum on DVE (1/NSAMP folded into wt host-side)
                acc = accp.tile([P, F], mybir.dt.float32, tag="acc")
                nc.vector.tensor_add(acc[:], slots[1][:], slots[2][:])
                for s in range(3, SLOTS):
                    nc.vector.tensor_add(acc[:], acc[:], slots[s][:])
                # transposes -> combT[kc][:, bt*128:(bt+1)*128]
                for kc, (src, c) in enumerate(
                        [(slots[0], 0), (slots[0], 1), (acc, 0), (acc, 1)]):
                    pt = pst.tile([P, P], mybir.dt.float32, tag="pt")
                    nc.tensor.transpose(out=pt[:], in_=src[:, c * P:(c + 1) * P],
                                        identity=ident[:])
                    nc.scalar.copy(out=combT[kc][:, bt * P:(bt + 1) * P], in_=pt[:])
                if bt == GROUP - 1:
                    for ec in range(2):
                        pm = psm.tile([P, GROUP * P], mybir.dt.float32, tag="pm")
                        for kc in range(4):
                            nc.tensor.matmul(
                                out=pm[:],
                                lhsT=wtile[:, kc * E + ec * P: kc * E + (ec + 1) * P],
                                rhs=combT[kc][:],
                                start=(kc == 0), stop=(kc == 3),
                            )
                        o = outsp.tile([P, GROUP * P], mybir.dt.float32, tag="o")
                        nc.scalar.activation(o[:], pm[:],
                                             mybir.ActivationFunctionType.Relu)
                        nc.sync.dma_start(
                            out=out.ap()[ec * P:(ec + 1) * P,
                                         g * GROUP * P:(g + 1) * GROUP * P],
                            in_=o[:])
    nc.compile()
    return nc


def _get_nc():
    if "nc" not in _cache:
        _cache["nc"] = _build()
    return _cache["nc"]


def run(features, W, nodes, neigh_idx, trace=False):
    from concourse.bass_utils import run_bass_kernel_spmd

    features = np.ascontiguousarray(np.asarray(features), dtype=np.float32)
    W = np.asarray(W, dtype=np.float32)
    nodes = np.asarray(nodes).astype(np.int32)
    neigh = np.asarray(neigh_idx).astype(np.int32)

    wt = np.ascontiguousarray(
        np.concatenate([W[:, :F].T, W[:, F:].T / NSAMP], axis=0), dtype=np.float32)

    in_maps = []
    for c in range(N_CORES):
        sl = slice(c * B_LOCAL, (c + 1) * B_LOCAL)
        gx = np.concatenate([nodes[sl, None], neigh[sl]], axis=1)
        in_maps.append({"features": features, "wt": wt,
                        "gidx": np.ascontiguousarray(gx, dtype=np.int32)})

    res = run_bass_kernel_spmd(_get_nc(), in_maps,
                               core_ids=list(range(N_CORES)), trace=trace)
    out = np.concatenate([r["out"] for r in res.results], axis=1)
    return out, res


def kernel(features, W, nodes, neigh_idx):
    out, _ = run(features, W, nodes, neigh_idx)
    return out
